# revision 4
# baseline (speedup 1.0000x reference)
"""Causal self-attention (B=4, T=2048, C=1024, H=16, D=64) on 8 TRN2 NeuronCores.

Sharding: core i handles batch b = i//2 and head-group g = i%2 (8 of the 16
heads).  Each core computes the QKV projection for its batch restricted to its
heads' columns, runs causal attention for its 8 heads, and produces a partial
output projection y_part = ctx_g @ w_out[rows of g].  The two partials per
batch are summed on the host (y[b] = y_part[2b] + y_part[2b+1]).

Per-core kernel layout choices:
  - x arrives pre-transposed from the host as x_t [C, T] so the contraction
    dim (C) sits on SBUF partitions for every matmul.
  - q,k,v are produced transposed ([channel, t]) in bf16; scores are computed
    transposed (scores_T[tk, tq]) so softmax normalization can ride on the PV
    matmul via a ones column appended to v, and no attention-weight transpose
    is ever needed.
  - exp() skips max-subtraction (scores for this distribution are |s| < ~10;
    raw exp is safe in fp32).
  - matmuls: fp32r (full PE rate at N>=256) for QKV / output projection, bf16
    for QK^T and PV.
"""

import numpy as np

import concourse.bass as bass
import concourse.mybir as mybir
from concourse import bacc, tile
from concourse.bass_utils import run_bass_kernel_spmd
from concourse.masks import make_identity

F32 = mybir.dt.float32
BF16 = mybir.dt.bfloat16
F32R = mybir.dt.float32r

B, T, C = 4, 2048, 1024
H, D = 16, 64
N_CORES = 8


def build_core_program(R=T, HPC=8, C_=C):
    """Build the single-core SPMD program.

    R: rows (sequence length) handled by this core.
    HPC: heads per core (even).
    """
    KC = C_ // 128            # contraction chunks for QKV matmul
    SUBS = HPC // 2           # 128-row groups per q/k/v section of qkv_T
    MC = 3 * SUBS             # 128-col chunks of this core's w_qkv slice
    CTXC = HPC * D            # ctx channels owned by this core
    OKC = CTXC // 128         # contraction chunks for out-proj
    NCH = R // 128            # tk/tq chunks
    TQ = min(512, R)          # qkv matmul moving width
    NT = R // TQ
    SW = min(1024, R)         # scores psum width (2 PSUM banks)
    EXP = mybir.ActivationFunctionType.Exp

    nc = bacc.Bacc("TRN2", target_bir_lowering=False, debug=False)

    x_t = nc.dram_tensor("x_t", [C_, R], F32R, kind="ExternalInput")
    w_qkv_c = nc.dram_tensor("w_qkv_c", [C_, 3 * CTXC], F32R, kind="ExternalInput")
    w_out_c = nc.dram_tensor("w_out_c", [CTXC, C_], F32R, kind="ExternalInput")
    y_part = nc.dram_tensor("y_part", [R, C_], F32, kind="ExternalOutput")

    with tile.TileContext(nc) as tc:
        with (
            tc.tile_pool(name="const", bufs=1) as constp,
            tc.tile_pool(name="qkv", bufs=1) as qkvp,
            tc.tile_pool(name="ctx", bufs=1) as ctxp,
        ):
            ident_f32 = constp.tile([128, 128], F32)
            make_identity(nc, ident_f32)
            ident_bf = constp.tile([128, 128], BF16)
            make_identity(nc, ident_bf)
            # tri[p, f] = 0 if f >= p else -1e9   (causal mask for the
            # diagonal 128x128 block of transposed scores)
            tri = constp.tile([128, 128], F32)
            nc.gpsimd.memset(tri, 0.0)
            nc.gpsimd.affine_select(
                out=tri, in_=tri,
                compare_op=mybir.AluOpType.is_ge,
                fill=-1e9, base=0,
                pattern=[[1, 128]], channel_multiplier=-1,
            )

            qT = qkvp.tile([128, SUBS, R], BF16)
            kT = qkvp.tile([128, SUBS, R], BF16)
            vT = qkvp.tile([128, SUBS, R], BF16)
            ctx_sb = ctxp.tile([128, NCH, CTXC], F32)

            # ---- Phase 1: qkv_T = w_qkv_c.T @ x_t  (fp32r) ----
            with (
                tc.tile_pool(name="wp", bufs=1) as wp,
                tc.tile_pool(name="xp", bufs=2) as xp,
                tc.tile_pool(name="qkvps", bufs=2, space="PSUM") as qps,
            ):
                w_sb = wp.tile([128, KC, 3 * CTXC], F32R)
                for kc in range(KC):
                    nc.sync.dma_start(
                        out=w_sb[:, kc, :],
                        in_=w_qkv_c[128 * kc:128 * (kc + 1), :],
                    )
                for n in range(NT):
                    x_sb = xp.tile([128, KC, TQ], F32R, name="x_sb", tag="x_sb")
                    for kc in range(KC):
                        nc.sync.dma_start(
                            out=x_sb[:, kc, :],
                            in_=x_t[128 * kc:128 * (kc + 1), n * TQ:(n + 1) * TQ],
                        )
                    for mc in range(MC):
                        ps = qps.tile([128, TQ], F32, name="qkv_ps", tag="qkv_ps")
                        for kc in range(KC):
                            nc.tensor.matmul(
                                ps,
                                lhsT=w_sb[:, kc, 128 * mc:128 * (mc + 1)],
                                rhs=x_sb[:, kc, :],
                                start=(kc == 0), stop=(kc == KC - 1),
                            )
                        sec, sub = mc // SUBS, mc % SUBS
                        dest = (qT, kT, vT)[sec]
                        nc.vector.tensor_copy(
                            out=dest[:, sub, n * TQ:(n + 1) * TQ], in_=ps
                        )

            # ---- Phases 2+3: attention, head pairs ----
            with (
                tc.tile_pool(name="attn", bufs=2) as attnp,
                tc.tile_pool(name="vp", bufs=2) as vp,
                tc.tile_pool(name="smallsb", bufs=4) as smallsb,
                tc.tile_pool(name="scoresps", bufs=2, space="PSUM") as sps,
                tc.tile_pool(name="smallps", bufs=2, space="PSUM") as smp,
            ):
                for sub in range(SUBS):
                    # v_sb: [ones | v_even | ones | v_odd] per tk chunk
                    # col 0: ones, 1..64: head 2*sub, 65: ones, 66..129: head 2*sub+1
                    v_sb = vp.tile([128, NCH, 130], BF16, name="v_sb", tag="v_sb")
                    nc.gpsimd.memset(v_sb[:, :, 0:1], 1.0)
                    nc.gpsimd.memset(v_sb[:, :, 65:66], 1.0)
                    for i in range(NCH):
                        tp = smp.tile([128, 128], BF16, name="vt_ps", tag="vt_ps")
                        nc.tensor.transpose(
                            tp, vT[:, sub, 128 * i:128 * (i + 1)], ident_bf
                        )
                        nc.vector.tensor_copy(out=v_sb[:, i, 1:65], in_=tp[:, 0:64])
                        nc.vector.tensor_copy(out=v_sb[:, i, 66:130], in_=tp[:, 64:128])

                    for hh in (2 * sub, 2 * sub + 1):
                        p0 = 64 * (hh % 2)
                        qh = qT[p0:p0 + 64, sub, :]
                        kh = kT[p0:p0 + 64, sub, :]
                        # scores_T chunk i covers tq in [128*i, R)
                        attn_tiles = []
                        for i in range(NCH):
                            W = R - 128 * i
                            at = attnp.tile([128, W], BF16,
                                            name=f"at{i}", tag=f"attn{i}")
                            off = 0
                            while off < W:
                                pw = min(SW, W - off)
                                ps = sps.tile([128, SW], F32, name="sc_ps",
                                              tag="sc_ps")
                                for p in range(0, pw, 512):
                                    nw = min(512, pw - p)
                                    q0 = 128 * i + off + p
                                    nc.tensor.matmul(
                                        ps[:, p:p + nw],
                                        lhsT=kh[:, 128 * i:128 * (i + 1)],
                                        rhs=qh[:, q0:q0 + nw],
                                        start=True, stop=True,
                                    )
                                if off == 0:
                                    nc.vector.tensor_add(
                                        ps[:, 0:128], ps[:, 0:128], tri
                                    )
                                nc.scalar.activation(
                                    at[:, off:off + pw], ps[:, :pw],
                                    EXP, scale=0.125,
                                )
                                off += pw
                            attn_tiles.append(at)
                        # PV: ctx[tq, :] with denominator in col 0
                        vcol = 65 * (hh % 2)
                        for j in range(NCH):
                            cps = smp.tile([128, 128], F32, name="sm_ps",
                                           tag="sm_ps")
                            for i in range(j + 1):
                                o = 128 * (j - i)
                                nc.tensor.matmul(
                                    cps[:, 0:65],
                                    lhsT=attn_tiles[i][:, o:o + 128],
                                    rhs=v_sb[:, i, vcol:vcol + 65],
                                    start=(i == 0), stop=(i == j),
                                )
                            rec = smallsb.tile([128, 1], F32, name="rec",
                                               tag="rec")
                            nc.vector.reciprocal(rec, cps[:, 0:1])
                            nc.vector.tensor_mul(
                                ctx_sb[:, j, D * hh:D * (hh + 1)],
                                cps[:, 1:65],
                                rec.to_broadcast((128, 64)),
                            )

            # ---- Phase 4: y_part = ctx @ w_out_c  (fp32r) ----
            with (
                tc.tile_pool(name="ctxT", bufs=1) as ctxTp,
                tc.tile_pool(name="wout", bufs=1) as woutp,
                tc.tile_pool(name="yev", bufs=3) as yevp,
                tc.tile_pool(name="tps", bufs=2, space="PSUM") as tps,
                tc.tile_pool(name="yps", bufs=2, space="PSUM") as yps,
            ):
                w_out_sb = woutp.tile([128, OKC, C_], F32R)
                for kc in range(OKC):
                    nc.sync.dma_start(
                        out=w_out_sb[:, kc, :],
                        in_=w_out_c[128 * kc:128 * (kc + 1), :],
                    )
                ctx_T = ctxTp.tile([128, OKC, R], F32R)
                for j in range(NCH):
                    for cc in range(OKC):
                        tp = tps.tile([128, 128], F32, name="t_ps", tag="t_ps")
                        nc.tensor.transpose(
                            tp, ctx_sb[:, j, 128 * cc:128 * (cc + 1)], ident_f32
                        )
                        nc.vector.tensor_copy(
                            out=ctx_T[:, cc, 128 * j:128 * (j + 1)], in_=tp
                        )
                for m in range(NCH):
                    for nn in range(C_ // 512):
                        yp = yps.tile([128, 512], F32, name="y_ps", tag="y_ps")
                        for kc in range(OKC):
                            nc.tensor.matmul(
                                yp,
                                lhsT=ctx_T[:, kc, 128 * m:128 * (m + 1)],
                                rhs=w_out_sb[:, kc, 512 * nn:512 * (nn + 1)],
                                start=(kc == 0), stop=(kc == OKC - 1),
                            )
                        ye = yevp.tile([128, 512], F32, name="ye", tag="ye")
                        nc.vector.tensor_copy(out=ye, in_=yp)
                        nc.sync.dma_start(
                            out=y_part[128 * m:128 * (m + 1),
                                       512 * nn:512 * (nn + 1)],
                            in_=ye,
                        )

    nc.finalize()
    return nc


def make_in_maps(x, w_qkv, w_out):
    x = np.asarray(x, dtype=np.float32)
    w_qkv = np.asarray(w_qkv, dtype=np.float32)
    w_out = np.asarray(w_out, dtype=np.float32)
    in_maps = []
    for core in range(N_CORES):
        b, g = core // 2, core % 2
        cols = slice(512 * g, 512 * (g + 1))
        wq = np.ascontiguousarray(
            np.concatenate(
                [w_qkv[:, cols], w_qkv[:, 1024:][:, cols], w_qkv[:, 2048:][:, cols]],
                axis=1,
            )
        )
        in_maps.append({
            "x_t": np.ascontiguousarray(x[b].T),
            "w_qkv_c": wq,
            "w_out_c": np.ascontiguousarray(w_out[512 * g:512 * (g + 1), :]),
        })
    return in_maps


_NC_CACHE = None
LAST_RESULT = None


def kernel(x, w_qkv, w_out):
    global _NC_CACHE, LAST_RESULT
    if _NC_CACHE is None:
        _NC_CACHE = build_core_program()
    nc = _NC_CACHE
    in_maps = make_in_maps(x, w_qkv, w_out)
    res = run_bass_kernel_spmd(nc, in_maps, list(range(N_CORES)))
    LAST_RESULT = res
    outs = [r["y_part"] for r in res.results]
    y = np.stack([outs[2 * b] + outs[2 * b + 1] for b in range(B)], axis=0)
    return y.astype(np.float32)
